# revision 1
# baseline (speedup 1.0000x reference)
"""DenseCRF loss kernel for Trainium2, data-parallel over batch on 8 NeuronCores.

reference:
  seg = bilinear_resize(segmentations, 128->64)            # [N,K,64,64]
  f_i = [x_i/50, y_i/50, r_i/15, g_i/15, b_i/15]           # 5-dim bilateral feature
  W_ij = exp(-0.5*|f_i - f_j|^2)                           # [P,P], P=4096
  loss = WEIGHT * (-sum_k s_k^T W s_k) / N

Per core (1 image): W block = exp(G - q_i - q_j) with G the 5-d Gram matrix.
G is computed on the TensorEngine as a 22-row bf16 matmul where every feature is
split hi/lo into two bf16 values (bf16 products are exact in the fp32 PSUM
accumulator, so the only error is the tiny split residual). -q_i rides two bf16
aux rows; -q_j is the fp32 per-partition bias of the Exp activation. The exp'd
block (bf16) is contracted against the resized segmentation with PSUM
accumulation; a DVE multiply+reduce forms the scalar, host sums 8 cores.

Row pairing of the 22-row contraction (FA row r pairs with FB row r):
  FA: [H5 | H5 | L5 | L5 | 1 1]     H5 = [pxh pyh fh_r fh_g fh_b]
  FB: [H5 | L5 | H5 | L5 | -qh -ql] L5 = [pxl pyl fl_r fl_g fl_b]
"""

import sys

sys.path.insert(0, "/opt/trn_rl_repo")

import numpy as np
import ml_dtypes

import concourse.bass as bass
import concourse.tile as tile
from concourse import bacc, bass_isa, mybir
from concourse.bass_utils import run_bass_kernel_spmd

F32 = mybir.dt.float32
BF16 = mybir.dt.bfloat16
AF = mybir.ActivationFunctionType
ALU = mybir.AluOpType
BF = ml_dtypes.bfloat16

N, C, K = 8, 3, 21
H, W = 64, 64
P = H * W  # 4096
SIGMA_RGB = 15.0
SXY = 100.0 * 0.5  # sigma_xy * scale
WEIGHT = 1e-8
NB = 32  # 128-row chunks of P
NG = 8  # 512-col groups of P


def _resize_matrix():
    """[64,128] weights of jax.image.resize(..., method='bilinear') along one dim
    (triangle kernel, antialias=True, scale=0.5, renormalized)."""
    y = np.arange(128, dtype=np.float64)[:, None]
    sample = 2.0 * np.arange(64, dtype=np.float64)[None, :] + 0.5
    w = np.maximum(0.0, 1.0 - 0.5 * np.abs(y - sample))
    w = w / w.sum(axis=0, keepdims=True)
    return np.ascontiguousarray(w.T.astype(np.float32))  # [64,128]


def _consts():
    R = _resize_matrix()  # [64,128]
    rtf = np.ascontiguousarray(R.T)  # [128,64] f32
    rtb = rtf.astype(BF)
    idf = np.eye(128, dtype=np.float32)
    idb = idf.astype(BF)
    i = np.arange(P, dtype=np.float32)
    px = (i % 64).astype(np.float32) / np.float32(SXY)
    py = (i // 64).astype(np.float32) / np.float32(SXY)
    pos = np.stack([px, py])  # [2,P] f32
    ph2 = pos.astype(BF)
    pl2 = (pos - ph2.astype(np.float32)).astype(BF)
    pf2 = ph2.astype(np.float32) + pl2.astype(np.float32)  # exact f~ for positions
    # constant skeletons of FA/FB: position + ones rows, zeros where the
    # color / q rows get DMA'd in on-device
    fabA = np.zeros((22, P), dtype=BF)
    fabB = np.zeros((22, P), dtype=BF)
    fabA[0:2] = ph2
    fabA[5:7] = ph2
    fabA[10:12] = pl2
    fabA[15:17] = pl2
    fabA[20:22] = np.ones((2, P), dtype=BF)
    fabB[0:2] = ph2
    fabB[10:12] = ph2
    fabB[5:7] = pl2
    fabB[15:17] = pl2
    neghalf5 = np.full((5, 1), -0.5, dtype=np.float32)
    return dict(rtf=rtf, rtb=rtb, idf=idf, idb=idb, fabA=fabA, fabB=fabB,
                pf2=pf2, neghalf5=neghalf5)


def _build():
    nc = bacc.Bacc()
    images_d = nc.dram_tensor("images", [C, H, W], F32, kind="ExternalInput")
    seg_d = nc.dram_tensor("segmentations", [K, 128, 128], F32, kind="ExternalInput")
    rtf_d = nc.dram_tensor("rtf", [128, 64], F32, kind="ExternalInput")
    rtb_d = nc.dram_tensor("rtb", [128, 64], BF16, kind="ExternalInput")
    idf_d = nc.dram_tensor("idf", [128, 128], F32, kind="ExternalInput")
    idb_d = nc.dram_tensor("idb", [128, 128], BF16, kind="ExternalInput")
    fabA_d = nc.dram_tensor("fabA", [22, P], BF16, kind="ExternalInput")
    fabB_d = nc.dram_tensor("fabB", [22, P], BF16, kind="ExternalInput")
    pf2_d = nc.dram_tensor("pf2", [2, P], F32, kind="ExternalInput")
    nh5_d = nc.dram_tensor("neghalf5", [5, 1], F32, kind="ExternalInput")
    out_d = nc.dram_tensor("out", [1], F32, kind="ExternalOutput")

    with tile.TileContext(nc) as tc:
        with tc.tile_pool(name="persist", bufs=1) as pp:
            FA = pp.tile([22, P], BF16, tag="FA")
            FB = pp.tile([22, P], BF16, tag="FB")
            qcol = pp.tile([128, NB], F32, tag="qcol")
            Ftil = pp.tile([5, P], F32, tag="Ftil")
            Fsq = pp.tile([5, P], F32, tag="Fsq")
            q2ar = pp.tile([5, P], F32, tag="q2ar")
            fh3 = pp.tile([3, P], BF16, tag="fh3")
            fl3 = pp.tile([3, P], BF16, tag="fl3")
            qh1 = pp.tile([1, P], BF16, tag="qh1")
            ql1 = pp.tile([1, P], BF16, tag="ql1")
            nh5_s = pp.tile([5, 1], F32, tag="nh5")
            Srow = pp.tile([K, P], F32, tag="Srow")
            STt = pp.tile([128, NB * K], BF16, tag="STt")
            rtf_s = pp.tile([128, 64], F32, tag="rtf")
            rtb_s = pp.tile([128, 64], BF16, tag="rtb")
            idf_s = pp.tile([128, 128], F32, tag="idf")
            idb_s = pp.tile([128, 128], BF16, tag="idb")
            img_s = pp.tile([C, P], F32, tag="img")
            seg_s = pp.tile([128, K * 128], F32, tag="seg")
            A_sb = pp.tile([64, K * 128], BF16, tag="A_sb")
            At = pp.tile([128, K * 64], BF16, tag="At")
            partials = pp.tile([K, NG], F32, tag="partials")
            pr1 = pp.tile([K, 1], F32, tag="pr1")
            tot = pp.tile([K, 1], F32, tag="tot")
            osb = pp.tile([1, 1], F32, tag="osb")

            # ---- load inputs / constants ----
            # DMA issue cost (~1.7us each) serializes per queue: spread over the
            # three DMA-capable queues. The q-chain (images -> colors -> Fsq ->
            # all-reduce -> qh/ql -> FB rows) is the critical path, so the Pool
            # queue carries only seg (the all-reduce must run early) and the
            # ACT queue runs its compute before its replica DMAs.
            dma = nc.sync.dma_start
            dmag = nc.gpsimd.dma_start
            dmaa = nc.scalar.dma_start
            inv15 = float(np.float32(1.0) / np.float32(SIGMA_RGB))
            # Queue layout: images first on the gpsimd queue (it gates the
            # q-chain), then resize inputs in usage order; FA/q-row writes ride
            # the sync queue tail; FB color replicas ride the ACT queue after
            # its compute.
            dmag(img_s[:], images_d.rearrange("c h w -> c (h w)"))
            dmag(seg_s[:], seg_d.rearrange("k y x -> y k x"))
            dmag(FB[:], fabB_d[:])
            dmag(rtf_s[:], rtf_d[:])
            dmag(idb_s[:], idb_d[:])
            dmag(rtb_s[:], rtb_d[:])
            dmag(idf_s[:], idf_d[:])
            dma(Ftil[3:5, :], pf2_d[:])
            dma(FA[:], fabA_d[:])
            dma(nh5_s[:], nh5_d[:])

            # color features (hi/lo split of img/15) at partition 0; engines
            # cannot address partition offsets that aren't multiples of 32, so
            # rows are staged and DMA'd into the FA/FB row slots.
            inv15 = float(np.float32(1.0) / np.float32(SIGMA_RGB))
            nc.scalar.activation(fh3[:], img_s[:], AF.Copy, scale=inv15)  # fh
            nc.scalar.activation(Ftil[0:3, :], img_s[:], AF.Copy, scale=inv15)
            nc.vector.scalar_tensor_tensor(
                fl3[:], img_s[:], inv15, fh3[:], ALU.mult, ALU.subtract
            )  # fl = img/15 - fh

            # q = 0.5|f~|^2  (Ftil rows: [colors | positions]; sum is order-free)
            nc.vector.tensor_mul(Fsq[:], Ftil[:], Ftil[:])
            nc.gpsimd.partition_all_reduce(q2ar[:], Fsq[:], 5, bass_isa.ReduceOp.add)
            q2row = q2ar[0:1, :]
            nc.scalar.activation(qh1[:], q2row, AF.Copy, scale=-0.5)  # -qh
            nc.vector.scalar_tensor_tensor(
                ql1[:], q2row, -0.5, qh1[:], ALU.mult, ALU.subtract
            )  # -ql = -q - (-qh)
            dma(FB[20:21, :], qh1[:])
            dma(FB[21:22, :], ql1[:])
            dmaa(FB[2:5, :], fh3[:])
            dmaa(FB[12:15, :], fh3[:])
            dmaa(FB[7:10, :], fl3[:])
            dmaa(FB[17:20, :], fl3[:])
            dma(FA[2:5, :], fh3[:])
            dma(FA[7:10, :], fh3[:])
            dma(FA[12:15, :], fl3[:])
            dma(FA[17:20, :], fl3[:])

            with tc.tile_pool(name="prep_ps", bufs=8, space="PSUM") as pps:
                # ---- resize: rows (contract Y) ----
                # (emitted before the q-dependent matmuls: PE executes in order,
                # and resize inputs arrive long before Fsq is ready)
                for c0 in range(0, K * 128, 512):
                    c1 = min(c0 + 512, K * 128)
                    aps = pps.tile([64, 512], F32, tag="ps", name=f"aps{c0}")
                    nc.tensor.matmul(
                        aps[:, : c1 - c0], rtf_s[:], seg_s[:, c0:c1],
                        start=True, stop=True,
                    )
                    nc.vector.tensor_copy(A_sb[:, c0:c1], aps[:, : c1 - c0])
                # transpose per class: At[X, (k,y')]
                for k0 in range(0, K, 8):
                    k1 = min(k0 + 8, K)
                    tps = pps.tile([128, 64 * 8], BF16, tag="ps", name=f"tps{k0}")
                    for k in range(k0, k1):
                        nc.tensor.transpose(
                            tps[:, (k - k0) * 64 : (k - k0 + 1) * 64],
                            A_sb[0:64, k * 128 : (k + 1) * 128], idb_s[0:64, 0:64]
                        )
                    nc.vector.tensor_copy(
                        At[:, k0 * 64 : k1 * 64], tps[:, : (k1 - k0) * 64]
                    )
                # cols (contract X): Srow[k, y'*64+x']
                at3 = At[:, :].rearrange("x (k y) -> x k y", k=K, y=64)
                for yb in range(8):
                    sps = pps.tile([K, 512], F32, tag="ps", name=f"sps{yb}")
                    for yl in range(8):
                        yp = yb * 8 + yl
                        nc.tensor.matmul(
                            sps[:, yl * 64 : (yl + 1) * 64],
                            at3[:, :, yp], rtb_s[:],
                            start=True, stop=True,
                        )
                    nc.vector.tensor_copy(Srow[:, yb * 512 : (yb + 1) * 512], sps[:])
                # STt chunks: [128, 21] per b (bf16, acc-matmul weights)
                for b0 in range(0, NB, 8):
                    t2 = pps.tile([128, K * 8], F32, tag="ps", name=f"t2_{b0}")
                    for b in range(b0, b0 + 8):
                        nc.tensor.transpose(
                            t2[:, (b - b0) * K : (b - b0 + 1) * K],
                            Srow[:, b * 128 : (b + 1) * 128], idf_s[0:K, 0:K]
                        )
                    nc.vector.tensor_copy(STt[:, b0 * K : (b0 + 8) * K], t2[:])

                # qcol[:, b] = -q for chunk b (fp32, used as Exp bias)
                qps = pps.tile([128, NB], F32, tag="ps", name="qps")
                for b in range(NB):
                    nc.tensor.matmul(
                        qps[:, b : b + 1],
                        Fsq[:, b * 128 : (b + 1) * 128],
                        nh5_s[:],
                        start=True, stop=True,
                    )
                nc.vector.tensor_copy(qcol[:], qps[:])


            # ---- main loop: 4 passes x 32 chunks x one 1024-wide exp unit ----
            with (
                tc.tile_pool(name="gps", bufs=3, space="PSUM") as gps,
                tc.tile_pool(name="accps", bufs=2, space="PSUM") as accps,
                tc.tile_pool(name="ep", bufs=8) as ep,
                tc.tile_pool(name="finp", bufs=2) as finp,
            ):
                for p in range(4):
                    accs = [
                        accps.tile([K, 512], F32, tag="acc", name=f"acc{p}_{gi}")
                        for gi in range(2)
                    ]
                    pend = []  # software pipeline: acc-matmuls lag one chunk
                    for b in range(NB):
                        fa_b = FA[:, b * 128 : (b + 1) * 128]
                        g0 = p * 2
                        if len(pend) > 1:
                            pb, pet = pend.pop(0)
                            for gi in range(2):
                                nc.tensor.matmul(
                                    accs[gi][:],
                                    STt[:, pb * K : (pb + 1) * K],
                                    pet[:, gi * 512 : (gi + 1) * 512],
                                    start=(pb == 0), stop=(pb == NB - 1),
                                )
                        gt = gps.tile([128, 1024], F32, tag="g", name=f"g{p}_{b}")
                        nc.tensor.matmul(
                            gt[:, 0:512], fa_b,
                            FB[:, g0 * 512 : (g0 + 1) * 512],
                            start=True, stop=True,
                        )
                        nc.tensor.matmul(
                            gt[:, 512:1024], fa_b,
                            FB[:, (g0 + 1) * 512 : (g0 + 2) * 512],
                            start=True, stop=True,
                        )
                        et = ep.tile([128, 1024], BF16, tag="e", name=f"e{p}_{b}")
                        nc.scalar.activation(
                            et[:], gt[:], AF.Exp, bias=qcol[:, b : b + 1]
                        )
                        pend.append((b, et))
                    for pb, pet in pend:
                        for gi in range(2):
                            nc.tensor.matmul(
                                accs[gi][:],
                                STt[:, pb * K : (pb + 1) * K],
                                pet[:, gi * 512 : (gi + 1) * 512],
                                start=(pb == 0), stop=(pb == NB - 1),
                            )
                    # loss partials: sum_k,i acc[k,i] * Srow[k,i]
                    for gi in range(2):
                        g = p * 2 + gi
                        sc = finp.tile([K, 512], F32, tag="sc", name=f"sc{p}_{gi}")
                        nc.vector.tensor_mul(
                            sc[:], accs[gi][:], Srow[:, g * 512 : (g + 1) * 512]
                        )
                        nc.vector.tensor_reduce(
                            partials[:, g : g + 1], sc[:], mybir.AxisListType.X, ALU.add
                        )

                nc.vector.tensor_reduce(pr1[:], partials[:], mybir.AxisListType.X, ALU.add)
                nc.gpsimd.partition_all_reduce(tot[:], pr1[:], K, bass_isa.ReduceOp.add)
                nc.scalar.activation(osb[:], tot[0:1, :], AF.Copy, scale=float(-WEIGHT / N))
                nc.sync.dma_start(out_d[:], osb[:])

    nc.finalize()
    return nc


_CACHE = {}


def _get_nc():
    if "nc" not in _CACHE:
        _CACHE["nc"] = _build()
    return _CACHE["nc"]


def kernel(images: np.ndarray, segmentations: np.ndarray) -> np.ndarray:
    images = np.ascontiguousarray(np.asarray(images, dtype=np.float32))
    segmentations = np.ascontiguousarray(np.asarray(segmentations, dtype=np.float32))
    assert images.shape == (N, C, H, W) and segmentations.shape == (N, K, 128, 128)
    nc = _get_nc()
    consts = _consts()
    in_maps = [
        {"images": images[n], "segmentations": segmentations[n], **consts}
        for n in range(N)
    ]
    res = run_bass_kernel_spmd(nc, in_maps, list(range(N)))
    total = sum(float(res.results[n]["out"][0]) for n in range(N))
    return np.array([total], dtype=np.float32)


if __name__ == "__main__":
    rng = np.random.RandomState(0)
    img = rng.rand(N, C, H, W).astype(np.float32) * 255.0
    seg = rng.rand(N, K, 128, 128).astype(np.float32)
    print(kernel(img, seg))



# revision 19
# speedup vs baseline: 1.7128x; 1.7128x over previous
"""DenseCRF loss kernel for Trainium2, data-parallel over batch on 8 NeuronCores.

reference:
  seg = bilinear_resize(segmentations, 128->64)            # [N,K,64,64]
  f_i = [x_i/50, y_i/50, r_i/15, g_i/15, b_i/15]           # 5-dim bilateral feature
  W_ij = exp(-0.5*|f_i - f_j|^2)                           # [P,P], P=4096
  loss = WEIGHT * (-sum_k s_k^T W s_k) / N

Per core (1 image). W is symmetric, so only the lower triangle at 512x512
block granularity is computed: col group g (512 cols) contracts row chunks
b >= 4g. Off-diagonal blocks count twice -- the x2 rides the Exp bias as an
exact fp32 +ln2 (exp(G+ln2) = 2 exp(G)).

G(i,j) = f_i.f_j - q_i - q_j (q = 0.5|f|^2) is one 24-row bf16 matmul:
features split hi/lo (products exact in fp32 PSUM), and BOTH -q_i and -q_j
ride hi/lo bf16 row pairs, so Exp needs no data bias and can batch any pair
of PSUM banks. Exp'd blocks (bf16) contract against the resized segmentation
with PSUM accumulation per col group; a fused DVE tensor_tensor_reduce forms
per-group partials; host sums 8 cores.

Row layout of FA/FB [28, P] (G += sum_r FA[r,i]*FB[r,j]):
  0-2  (ch,ch)  3-5 (ch,cl)  6-8 (cl,ch)  9-11 (cl,cl)     colors hi/lo
  12-13 FA=(-qch,-qcl) FB=1  14-15 FA=1 FB=(-qch,-qcl)     color-q rows
  16-17 (ph,ph) 18-19 (ph,pl) 20-21 (pl,ph) 22-23 (pl,pl)  positions (const)
  24-25 FA=(-qph,-qpl) FB=1  26-27 FA=1 FB=(-qph,-qpl)     position-q (const)
"""

import sys

sys.path.insert(0, "/opt/trn_rl_repo")

import numpy as np
import ml_dtypes

import concourse.bass as bass
import concourse.tile as tile
from concourse import bacc, bass_isa, mybir
from concourse.bass_utils import run_bass_kernel_spmd

F32 = mybir.dt.float32
F32R = mybir.dt.float32r
BF16 = mybir.dt.bfloat16
AF = mybir.ActivationFunctionType
ALU = mybir.AluOpType
BF = ml_dtypes.bfloat16

N, C, K = 8, 3, 21
H, W = 64, 64
P = H * W  # 4096
SIGMA_RGB = 15.0
SXY = 100.0 * 0.5  # sigma_xy * scale
WEIGHT = 1e-8
NB = 32  # 128-row chunks of P
NG = 8  # 512-col groups of P
NQ = 4  # 1024-col quarters (feature prep granularity)
LN2 = float(np.log(2.0))
KA, KB = 11, 10  # seg class split across the two load DMAs


def _resize_matrix():
    """[64,128] weights of jax.image.resize(..., method='bilinear') along one dim
    (triangle kernel, antialias=True, scale=0.5, renormalized)."""
    y = np.arange(128, dtype=np.float64)[:, None]
    sample = 2.0 * np.arange(64, dtype=np.float64)[None, :] + 0.5
    w = np.maximum(0.0, 1.0 - 0.5 * np.abs(y - sample))
    w = w / w.sum(axis=0, keepdims=True)
    return np.ascontiguousarray(w.T.astype(np.float32))  # [64,128]


def _consts():
    R = _resize_matrix()  # [64,128]
    rtf = np.ascontiguousarray(R.T)  # [128,64] f32
    rtb = rtf.astype(BF)
    idf = np.eye(K, dtype=np.float32)
    i = np.arange(P, dtype=np.float32)
    px = (i % 64).astype(np.float32) / np.float32(SXY)
    py = (i // 64).astype(np.float32) / np.float32(SXY)
    pos = np.stack([px, py])  # [2,P] f32
    ph2 = pos.astype(BF)
    pl2 = (pos - ph2.astype(np.float32)).astype(BF)
    pf2 = ph2.astype(np.float64) + pl2.astype(np.float64)  # exact f~ positions
    qpos = -0.5 * (pf2[0] ** 2 + pf2[1] ** 2)  # [P] f64
    qph = qpos.astype(np.float32).astype(BF)
    qpl = (qpos - qph.astype(np.float64)).astype(np.float32).astype(BF)
    # constant skeleton rows 12..27 of FA/FB (zeros where color-q rows land)
    skA = np.zeros((16, P), dtype=BF)
    skB = np.zeros((16, P), dtype=BF)
    skA[2:4] = 1.0
    skB[0:2] = 1.0
    skA[4:6] = ph2
    skA[6:8] = ph2
    skA[8:10] = pl2
    skA[10:12] = pl2
    skB[4:6] = ph2
    skB[6:8] = pl2
    skB[8:10] = ph2
    skB[10:12] = pl2
    skA[12], skA[13], skA[14], skA[15] = qph, qpl, 1.0, 1.0
    skB[12], skB[13], skB[14], skB[15] = 1.0, 1.0, qph, qpl
    return dict(rtf=rtf, rtb=rtb, idf=idf, fabA=skA, fabB=skB)


def _build():
    nc = bacc.Bacc()
    images_d = nc.dram_tensor("images", [C, P], F32, kind="ExternalInput")
    seg_d = nc.dram_tensor("segmentations", [K, 128, 128], F32, kind="ExternalInput")
    rtf_d = nc.dram_tensor("rtf", [128, 64], F32, kind="ExternalInput")
    rtb_d = nc.dram_tensor("rtb", [128, 64], BF16, kind="ExternalInput")
    idf_d = nc.dram_tensor("idf", [K, K], F32, kind="ExternalInput")
    fabA_d = nc.dram_tensor("fabA", [16, P], BF16, kind="ExternalInput")
    fabB_d = nc.dram_tensor("fabB", [16, P], BF16, kind="ExternalInput")
    out_d = nc.dram_tensor("out", [1], F32, kind="ExternalOutput")

    inv15 = float(np.float32(1.0) / np.float32(SIGMA_RGB))
    inv225 = float(np.float32(inv15) * np.float32(inv15))

    with tile.TileContext(nc) as tc:
        with (
            tc.tile_pool(name="persist", bufs=1) as pp,
            tc.tile_pool(name="rp", bufs=2, space="PSUM") as rp,
            tc.tile_pool(name="gps", bufs=2, space="PSUM") as gps,
            tc.tile_pool(name="accps", bufs=2, space="PSUM") as accps,
            tc.tile_pool(name="ep", bufs=10) as ep,
            tc.tile_pool(name="dscp", bufs=2) as dscp,
        ):
            FAq = [pp.tile([28, 1024], BF16, tag=f"FA{q}", name=f"FA{q}") for q in range(NQ)]
            FBq = [pp.tile([28, 1024], BF16, tag=f"FB{q}", name=f"FB{q}") for q in range(NQ)]
            img_s = pp.tile([C, P], F32, tag="img")
            seg_a = pp.tile([128, KA * 128], F32, tag="sega")
            seg_b = pp.tile([128, KB * 128], F32, tag="segb")
            rtf_s = pp.tile([128, 64], F32, tag="rtf")
            rtb_s = pp.tile([128, 64], BF16, tag="rtb")
            idf_s = pp.tile([K, K], F32, tag="idf")
            fsqq = [pp.tile([C, 1024], F32, tag=f"fsq{q}", name=f"fsq{q}") for q in range(NQ)]
            q3q = [pp.tile([C, 1024], F32, tag=f"q3{q}", name=f"q3{q}") for q in range(NQ)]
            cstq = [pp.tile([64, 1024], BF16, tag=f"cst{q}", name=f"cst{q}") for q in range(NQ)]
            cst2q = [pp.tile([64, 1024], BF16, tag=f"cs2{q}", name=f"cs2{q}") for q in range(NQ)]
            qstq = [pp.tile([64, 1024], BF16, tag=f"qst{q}", name=f"qst{q}") for q in range(NQ)]
            At = pp.tile([128, K * 64], BF16, tag="At")
            Srow_y = [pp.tile([K, 512], F32, tag=f"sr{y}", name=f"sr{y}") for y in range(NG)]
            STtb = [pp.tile([128, 8 * K], BF16, tag=f"stt{i}", name=f"stt{i}") for i in range(4)]
            partials = pp.tile([K, NG], F32, tag="partials")
            pr1 = pp.tile([K, 1], F32, tag="pr1")
            tot = pp.tile([K, 1], F32, tag="tot")
            osb = pp.tile([1, 1], F32, tag="osb")
            bln2 = pp.tile([128, 1], F32, tag="bln2")

            qS = nc.sync.dma_start
            qP = nc.gpsimd.dma_start
            qA = nc.scalar.dma_start

            # ---- input loads ----
            # SP: img first (gates the feature chain), then seg half A +
            # Q3 skeletons + resize consts. Pool: seg half B issued early
            # (transfer overlaps). Act: Q2 skeletons (idle early).
            nc.gpsimd.memset(bln2[:], LN2)
            qS(img_s[:], images_d[:])
            segr = seg_d.rearrange("k y x -> y k x")
            qP(seg_b[:], segr[:, KA:, :])
            qS(seg_a[:], segr[:, :KA, :])
            qS(rtf_s[:], rtf_d[:])
            qS(FAq[3][12:28, :], fabA_d[:, 3 * 1024 : 4 * 1024])
            qS(FBq[3][12:28, :], fabB_d[:, 3 * 1024 : 4 * 1024])
            qA(FAq[2][12:28, :], fabA_d[:, 2 * 1024 : 3 * 1024])
            qA(FBq[2][12:28, :], fabB_d[:, 2 * 1024 : 3 * 1024])
            qS(rtb_s[:], rtb_d[:])
            qS(idf_s[:], idf_d[:])

            def late_skels():
                for q in (1, 0):
                    sel = slice(q * 1024, (q + 1) * 1024)
                    qP(FAq[q][12:28, :], fabA_d[:, sel])
                    qP(FBq[q][12:28, :], fabB_d[:, sel])

            def feat_pre(q, fh_eng="act"):
                """Colors hi/lo for quarter q staged into cstq[q] quadrants
                (fh at 0, fh-copy at 32, fl at 64, fl-copy at 96), then ONE
                DMA each into FA[0:12] / FB[0:12] via strided partition APs.
                Pairing: FA rows = (h, h', l, l'), FB rows = (h, l, h', l')
                -> combos (h,h),(h,l),(l,h),(l,l)."""
                sel = slice(q * 1024, (q + 1) * 1024)
                cst, fsq = cstq[q], fsqq[q]
                fh, fl = cst[0:3, :], cst[32:35, :]
                if fh_eng == "act":
                    nc.scalar.activation(fh, img_s[:, sel], AF.Copy, scale=inv15)
                elif fh_eng == "dve":
                    nc.vector.tensor_scalar_mul(fh, img_s[:, sel], inv15)
                else:
                    nc.gpsimd.tensor_scalar_mul(fh, img_s[:, sel], inv15)
                nc.vector.scalar_tensor_tensor(
                    fsq[:], img_s[:, sel], inv225, img_s[:, sel], ALU.mult, ALU.mult
                )
                nc.vector.scalar_tensor_tensor(
                    fl, img_s[:, sel], inv15, fh, ALU.mult, ALU.subtract
                )
                FA, FB = FAq[q], FBq[q]
                moves = [
                    (FA[0:3, :], fh), (FB[0:3, :], fh),
                    (FA[6:9, :], fl), (FB[3:6, :], fl),
                    (FA[3:6, :], fh), (FB[6:9, :], fh),
                    (FA[9:12, :], fl), (FB[9:12, :], fl),
                ]
                for i, (dst, srct) in enumerate(moves):
                    [qS, qP][i % 2](dst, srct[:])

            def feat_post(q, qh_eng="act"):
                """color-q rows (-0.5|c|^2 hi/lo) staged into qstq[q]
                (qh at 0, ql at 32), one DMA each into FA[12:14]/FB[14:16]."""
                q3, qst = q3q[q], qstq[q]
                qh, ql = qst[0:1, :], qst[32:33, :]
                nc.gpsimd.partition_all_reduce(q3[:], fsqq[q][:], C, bass_isa.ReduceOp.add)
                if qh_eng == "act":
                    nc.scalar.activation(qh, q3[0:1, :], AF.Copy, scale=-0.5)
                elif qh_eng == "dve":
                    nc.vector.tensor_scalar_mul(qh, q3[0:1, :], -0.5)
                else:
                    nc.gpsimd.tensor_scalar_mul(qh, q3[0:1, :], -0.5)
                nc.vector.scalar_tensor_tensor(
                    ql, q3[0:1, :], -0.5, qh, ALU.mult, ALU.subtract
                )
                for i, (dst, srct) in enumerate([
                    (FAq[q][12:13, :], qh), (FBq[q][14:15, :], qh),
                    (FAq[q][13:14, :], ql), (FBq[q][15:16, :], ql),
                ]):
                    [qS, qP][i % 2](dst, srct)

            def at_stage():
                # At[x, k*64+y'] = sum_y seg[y,(k,x)] * rtf[y,y']  (f32r)
                for k0 in range(0, K, 8):
                    k1 = min(k0 + 8, K)
                    aps = rp.tile([128, 512], F32, tag="rp", name=f"at{k0}")
                    for k in range(k0, k1):
                        src = seg_a if k < KA else seg_b
                        koff = k if k < KA else k - KA
                        nc.tensor.matmul(
                            aps[:, (k - k0) * 64 : (k - k0 + 1) * 64],
                            src[:, koff * 128 : (koff + 1) * 128],
                            rtf_s[:],
                            start=True, stop=True,
                        )
                    nc.vector.tensor_copy(At[:, k0 * 64 : k1 * 64], aps[:, : (k1 - k0) * 64])

            at3 = None

            def srow_stage(ybs):
                # Srow[k, yb*512 + yl*64 + x'] = sum_x At[x,(k,y')] * rtb[x,x']
                for yb in ybs:
                    sps = rp.tile([128, 512], F32, tag="rp", name=f"sr{yb}")
                    for yl in range(8):
                        yp = yb * 8 + yl
                        nc.tensor.matmul(
                            sps[0:K, yl * 64 : (yl + 1) * 64],
                            at3[:, :, yp], rtb_s[:],
                            start=True, stop=True,
                        )
                    nc.vector.tensor_copy(Srow_y[yb][:], sps[0:K, :])

            def stt_stage(bi):
                # STt chunks for batch bi: chunks 8*bi .. 8*bi+7
                tps = rp.tile([128, 512], F32, tag="rp", name=f"st{bi}")
                for j in range(8):
                    b = 8 * bi + j
                    yb, rest = divmod(b * 128, 512)
                    nc.tensor.transpose(
                        tps[:, j * K : (j + 1) * K],
                        Srow_y[yb][:, rest : rest + 128],
                        idf_s[:],
                    )
                nc.vector.tensor_copy(STtb[bi][:], tps[:, : 8 * K])

            def group(g, defer_acc=False):
                """Col group g: G matmuls + Exp per chunk pair; acc matmuls
                accumulate S^T E; fused DVE dot forms partials[:, g]."""
                fbv = FBq[g // 2][:, (g % 2) * 512 : (g % 2 + 1) * 512]
                chunks = list(range(NB - 1, 4 * g - 1, -1))
                pairs = [(chunks[i], chunks[i + 1]) for i in range(0, len(chunks), 2)]
                acc = accps.tile([K, 512], F32, tag="acc", name=f"acc{g}")
                deferred = []

                def acc_mms(pair, et):
                    for j, b in enumerate(pair):
                        nc.tensor.matmul(
                            acc[:],
                            STtb[b // 8][:, (b % 8) * K : (b % 8 + 1) * K],
                            et[:, j * 512 : (j + 1) * 512],
                            start=(b == NB - 1), stop=(b == 4 * g),
                        )

                for pi, pair in enumerate(pairs):
                    gt = gps.tile([128, 1024], F32, tag="g", name=f"g{g}_{pi}")
                    for j, b in enumerate(pair):
                        nc.tensor.matmul(
                            gt[:, j * 512 : (j + 1) * 512],
                            FAq[b // 8][:, (b % 8) * 128 : (b % 8 + 1) * 128],
                            fbv,
                            start=True, stop=True,
                        )
                    et = ep.tile([128, 1024], BF16, tag="e", name=f"e{g}_{pi}")
                    diag = pair[0] < 4 * g + 4
                    nc.scalar.activation(et[:], gt[:], AF.Exp, bias=0.0 if diag else bln2[:])
                    if defer_acc:
                        deferred.append((pair, et))
                    else:
                        acc_mms(pair, et)
                return acc, deferred, acc_mms

            def dot(g, acc):
                dsc = dscp.tile([K, 512], F32, tag="dsc", name=f"dsc{g}")
                nc.vector.tensor_mul(dsc[:], acc[:], Srow_y[g][:])
                nc.vector.tensor_reduce(
                    partials[:, g : g + 1], dsc[:], mybir.AxisListType.X, ALU.add
                )

            # ---- emission schedule ----
            feat_pre(3, "act")
            feat_post(3, "act")
            at_stage()
            at3 = At[:, :].rearrange("x (k y) -> x k y", k=K, y=64)
            feat_pre(2, "act")
            feat_post(2, "dve")

            acc7, def7, accm7 = group(7, defer_acc=True)
            acc6, def6, accm6 = group(6, defer_acc=True)
            srow_stage([7, 6])
            stt_stage(3)
            for pair, et in def7:
                accm7(pair, et)
            dot(7, acc7)
            for pair, et in def6:
                accm6(pair, et)
            dot(6, acc6)

            acc5, def5, accm5 = group(5, defer_acc=True)
            srow_stage([5, 4])
            stt_stage(2)
            for pair, et in def5:
                accm5(pair, et)
            dot(5, acc5)

            acc4, _, _ = group(4)
            dot(4, acc4)
            late_skels()
            feat_pre(1, "pool")
            feat_post(1, "pool")
            srow_stage([3, 2])
            stt_stage(1)
            acc3, _, _ = group(3)
            dot(3, acc3)
            feat_pre(0, "pool")
            feat_post(0, "pool")
            srow_stage([1, 0])
            stt_stage(0)
            acc2, _, _ = group(2)
            dot(2, acc2)
            acc1, _, _ = group(1)
            dot(1, acc1)
            acc0, _, _ = group(0)
            dot(0, acc0)

            # ---- tail: sum partials, all-reduce over classes, scale ----
            nc.vector.tensor_reduce(pr1[:], partials[:], mybir.AxisListType.X, ALU.add)
            nc.gpsimd.partition_all_reduce(tot[:], pr1[:], K, bass_isa.ReduceOp.add)
            nc.vector.tensor_scalar_mul(osb[:], tot[0:1, :], float(-WEIGHT / N))
            nc.sync.dma_start(out_d[:], osb[:])

    nc.finalize()
    return nc


_CACHE = {}


def _get_nc():
    if "nc" not in _CACHE:
        _CACHE["nc"] = _build()
    return _CACHE["nc"]


def kernel(images: np.ndarray, segmentations: np.ndarray) -> np.ndarray:
    images = np.ascontiguousarray(np.asarray(images, dtype=np.float32))
    segmentations = np.ascontiguousarray(np.asarray(segmentations, dtype=np.float32))
    assert images.shape == (N, C, H, W) and segmentations.shape == (N, K, 128, 128)
    nc = _get_nc()
    consts = _consts()
    in_maps = [
        {"images": images[n].reshape(C, P), "segmentations": segmentations[n], **consts}
        for n in range(N)
    ]
    res = run_bass_kernel_spmd(nc, in_maps, list(range(N)))
    total = sum(float(res.results[n]["out"][0]) for n in range(N))
    return np.array([total], dtype=np.float32)


if __name__ == "__main__":
    rng = np.random.RandomState(0)
    img = rng.rand(N, C, H, W).astype(np.float32) * 255.0
    seg = rng.rand(N, K, 128, 128).astype(np.float32)
    print(kernel(img, seg))


# revision 21
# speedup vs baseline: 1.7166x; 1.0022x over previous
"""DenseCRF loss kernel for Trainium2, data-parallel over batch on 8 NeuronCores.

reference:
  seg = bilinear_resize(segmentations, 128->64)            # [N,K,64,64]
  f_i = [x_i/50, y_i/50, r_i/15, g_i/15, b_i/15]           # 5-dim bilateral feature
  W_ij = exp(-0.5*|f_i - f_j|^2)                           # [P,P], P=4096
  loss = WEIGHT * (-sum_k s_k^T W s_k) / N

Per core (1 image). W is symmetric, so only the lower triangle at 512x512
block granularity is computed: col group g (512 cols) contracts row chunks
b >= 4g. Off-diagonal blocks count twice -- the x2 rides the Exp bias as an
exact fp32 +ln2 (exp(G+ln2) = 2 exp(G)).

G(i,j) = f_i.f_j - q_i - q_j (q = 0.5|f|^2) is one 24-row bf16 matmul:
features split hi/lo (products exact in fp32 PSUM), and BOTH -q_i and -q_j
ride hi/lo bf16 row pairs, so Exp needs no data bias and can batch any pair
of PSUM banks. Exp'd blocks (bf16) contract against the resized segmentation
with PSUM accumulation per col group; a fused DVE tensor_tensor_reduce forms
per-group partials; host sums 8 cores.

Row layout of FA/FB [28, P] (G += sum_r FA[r,i]*FB[r,j]):
  0-2  (ch,ch)  3-5 (ch,cl)  6-8 (cl,ch)  9-11 (cl,cl)     colors hi/lo
  12-13 FA=(-qch,-qcl) FB=1  14-15 FA=1 FB=(-qch,-qcl)     color-q rows
  16-17 (ph,ph) 18-19 (ph,pl) 20-21 (pl,ph) 22-23 (pl,pl)  positions (const)
  24-25 FA=(-qph,-qpl) FB=1  26-27 FA=1 FB=(-qph,-qpl)     position-q (const)
"""

import sys

sys.path.insert(0, "/opt/trn_rl_repo")

import numpy as np
import ml_dtypes

import concourse.bass as bass
import concourse.tile as tile
from concourse import bacc, bass_isa, mybir
from concourse.bass_utils import run_bass_kernel_spmd

F32 = mybir.dt.float32
F32R = mybir.dt.float32r
BF16 = mybir.dt.bfloat16
AF = mybir.ActivationFunctionType
ALU = mybir.AluOpType
BF = ml_dtypes.bfloat16

N, C, K = 8, 3, 21
H, W = 64, 64
P = H * W  # 4096
SIGMA_RGB = 15.0
SXY = 100.0 * 0.5  # sigma_xy * scale
WEIGHT = 1e-8
NB = 32  # 128-row chunks of P
NG = 8  # 512-col groups of P
NQ = 4  # 1024-col quarters (feature prep granularity)
LN2 = float(np.log(2.0))
KA, KB = 11, 10  # seg class split across the two load DMAs
IMG_SHAPE = (C, 32, 128)  # img DRAM layout: 512B runs give sane DMA descriptors


def _resize_matrix():
    """[64,128] weights of jax.image.resize(..., method='bilinear') along one dim
    (triangle kernel, antialias=True, scale=0.5, renormalized)."""
    y = np.arange(128, dtype=np.float64)[:, None]
    sample = 2.0 * np.arange(64, dtype=np.float64)[None, :] + 0.5
    w = np.maximum(0.0, 1.0 - 0.5 * np.abs(y - sample))
    w = w / w.sum(axis=0, keepdims=True)
    return np.ascontiguousarray(w.T.astype(np.float32))  # [64,128]


def _consts():
    R = _resize_matrix()  # [64,128]
    rtf = np.ascontiguousarray(R.T)  # [128,64] f32
    rtb = rtf.astype(BF)
    idf = np.eye(K, dtype=np.float32)
    i = np.arange(P, dtype=np.float32)
    px = (i % 64).astype(np.float32) / np.float32(SXY)
    py = (i // 64).astype(np.float32) / np.float32(SXY)
    pos = np.stack([px, py])  # [2,P] f32
    ph2 = pos.astype(BF)
    pl2 = (pos - ph2.astype(np.float32)).astype(BF)
    pf2 = ph2.astype(np.float64) + pl2.astype(np.float64)  # exact f~ positions
    qpos = -0.5 * (pf2[0] ** 2 + pf2[1] ** 2)  # [P] f64
    qph = qpos.astype(np.float32).astype(BF)
    qpl = (qpos - qph.astype(np.float64)).astype(np.float32).astype(BF)
    # constant skeleton rows 12..27 of FA/FB (zeros where color-q rows land)
    skA = np.zeros((16, P), dtype=BF)
    skB = np.zeros((16, P), dtype=BF)
    skA[2:4] = 1.0
    skB[0:2] = 1.0
    skA[4:6] = ph2
    skA[6:8] = ph2
    skA[8:10] = pl2
    skA[10:12] = pl2
    skB[4:6] = ph2
    skB[6:8] = pl2
    skB[8:10] = ph2
    skB[10:12] = pl2
    skA[12], skA[13], skA[14], skA[15] = qph, qpl, 1.0, 1.0
    skB[12], skB[13], skB[14], skB[15] = 1.0, 1.0, qph, qpl
    return dict(rtf=rtf, rtb=rtb, idf=idf, fabA=skA, fabB=skB)


def _build():
    nc = bacc.Bacc()
    images_d = nc.dram_tensor("images", list(IMG_SHAPE), F32, kind="ExternalInput")
    seg_d = nc.dram_tensor("segmentations", [K, 128, 128], F32, kind="ExternalInput")
    rtf_d = nc.dram_tensor("rtf", [128, 64], F32, kind="ExternalInput")
    rtb_d = nc.dram_tensor("rtb", [128, 64], BF16, kind="ExternalInput")
    idf_d = nc.dram_tensor("idf", [K, K], F32, kind="ExternalInput")
    fabA_d = nc.dram_tensor("fabA", [16, P], BF16, kind="ExternalInput")
    fabB_d = nc.dram_tensor("fabB", [16, P], BF16, kind="ExternalInput")
    out_d = nc.dram_tensor("out", [1], F32, kind="ExternalOutput")

    inv15 = float(np.float32(1.0) / np.float32(SIGMA_RGB))
    inv225 = float(np.float32(inv15) * np.float32(inv15))

    with tile.TileContext(nc) as tc:
        with (
            tc.tile_pool(name="persist", bufs=1) as pp,
            tc.tile_pool(name="rp", bufs=2, space="PSUM") as rp,
            tc.tile_pool(name="gps", bufs=2, space="PSUM") as gps,
            tc.tile_pool(name="accps", bufs=2, space="PSUM") as accps,
            tc.tile_pool(name="ep", bufs=10) as ep,
            tc.tile_pool(name="dscp", bufs=2) as dscp,
        ):
            FAq = [pp.tile([28, 1024], BF16, tag=f"FA{q}", name=f"FA{q}") for q in range(NQ)]
            FBq = [pp.tile([28, 1024], BF16, tag=f"FB{q}", name=f"FB{q}") for q in range(NQ)]
            img_s = pp.tile([C, P], F32, tag="img")
            seg_a = pp.tile([128, KA * 128], F32, tag="sega")
            seg_b = pp.tile([128, KB * 128], F32, tag="segb")
            rtf_s = pp.tile([128, 64], F32, tag="rtf")
            rtb_s = pp.tile([128, 64], BF16, tag="rtb")
            idf_s = pp.tile([K, K], F32, tag="idf")
            fsqq = [pp.tile([C, 1024], F32, tag=f"fsq{q}", name=f"fsq{q}") for q in range(NQ)]
            q3q = [pp.tile([C, 1024], F32, tag=f"q3{q}", name=f"q3{q}") for q in range(NQ)]
            cstq = [pp.tile([64, 1024], BF16, tag=f"cst{q}", name=f"cst{q}") for q in range(NQ)]
            cst2q = [pp.tile([64, 1024], BF16, tag=f"cs2{q}", name=f"cs2{q}") for q in range(NQ)]
            qstq = [pp.tile([64, 1024], BF16, tag=f"qst{q}", name=f"qst{q}") for q in range(NQ)]
            At = pp.tile([128, K * 64], BF16, tag="At")
            Srow_y = [pp.tile([K, 512], F32, tag=f"sr{y}", name=f"sr{y}") for y in range(NG)]
            STtb = [pp.tile([128, 8 * K], BF16, tag=f"stt{i}", name=f"stt{i}") for i in range(4)]
            partials = pp.tile([K, NG], F32, tag="partials")
            pr1 = pp.tile([K, 1], F32, tag="pr1")
            tot = pp.tile([K, 1], F32, tag="tot")
            osb = pp.tile([1, 1], F32, tag="osb")
            bln2 = pp.tile([128, 1], F32, tag="bln2")

            qS = nc.sync.dma_start
            qP = nc.gpsimd.dma_start
            qA = nc.scalar.dma_start

            # ---- input loads ----
            # SP: img first (gates the feature chain), then seg half A +
            # Q3 skeletons + resize consts. Pool: seg half B issued early
            # (transfer overlaps). Act: Q2 skeletons (idle early).
            nc.gpsimd.memset(bln2[:], LN2)
            qP(img_s[:], images_d[:])  # [C,32,128] -> [C,P]
            segr = seg_d.rearrange("k y x -> y k x")
            qS(seg_a[:], segr[:, :KA, :])
            qP(seg_b[:], segr[:, KA:, :])
            qS(rtf_s[:], rtf_d[:])
            qA(FAq[3][12:28, :], fabA_d[:, 3 * 1024 : 4 * 1024])
            qA(FBq[3][12:28, :], fabB_d[:, 3 * 1024 : 4 * 1024])
            qA(FAq[2][12:28, :], fabA_d[:, 2 * 1024 : 3 * 1024])
            qA(FBq[2][12:28, :], fabB_d[:, 2 * 1024 : 3 * 1024])
            qA(rtb_s[:], rtb_d[:])
            qA(idf_s[:], idf_d[:])

            def late_skels():
                for q in (1, 0):
                    sel = slice(q * 1024, (q + 1) * 1024)
                    qP(FAq[q][12:28, :], fabA_d[:, sel])
                    qP(FBq[q][12:28, :], fabB_d[:, sel])

            def feat_pre(q, fh_eng="act"):
                """Colors hi/lo for quarter q staged into cstq[q] quadrants
                (fh at 0, fh-copy at 32, fl at 64, fl-copy at 96), then ONE
                DMA each into FA[0:12] / FB[0:12] via strided partition APs.
                Pairing: FA rows = (h, h', l, l'), FB rows = (h, l, h', l')
                -> combos (h,h),(h,l),(l,h),(l,l)."""
                sel = slice(q * 1024, (q + 1) * 1024)
                cst, fsq = cstq[q], fsqq[q]
                fh, fl = cst[0:3, :], cst[32:35, :]
                if fh_eng == "act":
                    nc.scalar.activation(fh, img_s[:, sel], AF.Copy, scale=inv15)
                elif fh_eng == "dve":
                    nc.vector.tensor_scalar_mul(fh, img_s[:, sel], inv15)
                else:
                    nc.gpsimd.tensor_scalar_mul(fh, img_s[:, sel], inv15)
                nc.vector.scalar_tensor_tensor(
                    fsq[:], img_s[:, sel], inv225, img_s[:, sel], ALU.mult, ALU.mult
                )
                nc.vector.scalar_tensor_tensor(
                    fl, img_s[:, sel], inv15, fh, ALU.mult, ALU.subtract
                )
                FA, FB = FAq[q], FBq[q]
                moves = [
                    (FA[0:3, :], fh), (FB[0:3, :], fh),
                    (FA[6:9, :], fl), (FB[3:6, :], fl),
                    (FA[3:6, :], fh), (FB[6:9, :], fh),
                    (FA[9:12, :], fl), (FB[9:12, :], fl),
                ]
                for i, (dst, srct) in enumerate(moves):
                    [qS, qP][i % 2](dst, srct[:])

            def feat_post(q, qh_eng="act"):
                """color-q rows (-0.5|c|^2 hi/lo) staged into qstq[q]
                (qh at 0, ql at 32), one DMA each into FA[12:14]/FB[14:16]."""
                q3, qst = q3q[q], qstq[q]
                qh, ql = qst[0:1, :], qst[32:33, :]
                nc.gpsimd.partition_all_reduce(q3[:], fsqq[q][:], C, bass_isa.ReduceOp.add)
                if qh_eng == "act":
                    nc.scalar.activation(qh, q3[0:1, :], AF.Copy, scale=-0.5)
                elif qh_eng == "dve":
                    nc.vector.tensor_scalar_mul(qh, q3[0:1, :], -0.5)
                else:
                    nc.gpsimd.tensor_scalar_mul(qh, q3[0:1, :], -0.5)
                nc.vector.scalar_tensor_tensor(
                    ql, q3[0:1, :], -0.5, qh, ALU.mult, ALU.subtract
                )
                for i, (dst, srct) in enumerate([
                    (FAq[q][12:13, :], qh), (FBq[q][14:15, :], qh),
                    (FAq[q][13:14, :], ql), (FBq[q][15:16, :], ql),
                ]):
                    [qS, qP][i % 2](dst, srct)

            def at_stage():
                # At[x, k*64+y'] = sum_y seg[y,(k,x)] * rtf[y,y']  (f32r)
                for k0 in range(0, K, 8):
                    k1 = min(k0 + 8, K)
                    aps = rp.tile([128, 512], F32, tag="rp", name=f"at{k0}")
                    for k in range(k0, k1):
                        src = seg_a if k < KA else seg_b
                        koff = k if k < KA else k - KA
                        nc.tensor.matmul(
                            aps[:, (k - k0) * 64 : (k - k0 + 1) * 64],
                            src[:, koff * 128 : (koff + 1) * 128],
                            rtf_s[:],
                            start=True, stop=True,
                        )
                    nc.vector.tensor_copy(At[:, k0 * 64 : k1 * 64], aps[:, : (k1 - k0) * 64])

            at3 = None

            def srow_stage(ybs):
                # Srow[k, yb*512 + yl*64 + x'] = sum_x At[x,(k,y')] * rtb[x,x']
                for yb in ybs:
                    sps = rp.tile([128, 512], F32, tag="rp", name=f"sr{yb}")
                    for yl in range(8):
                        yp = yb * 8 + yl
                        nc.tensor.matmul(
                            sps[0:K, yl * 64 : (yl + 1) * 64],
                            at3[:, :, yp], rtb_s[:],
                            start=True, stop=True,
                        )
                    nc.vector.tensor_copy(Srow_y[yb][:], sps[0:K, :])

            def stt_stage(bi):
                # STt chunks for batch bi: chunks 8*bi .. 8*bi+7
                tps = rp.tile([128, 512], F32, tag="rp", name=f"st{bi}")
                for j in range(8):
                    b = 8 * bi + j
                    yb, rest = divmod(b * 128, 512)
                    nc.tensor.transpose(
                        tps[:, j * K : (j + 1) * K],
                        Srow_y[yb][:, rest : rest + 128],
                        idf_s[:],
                    )
                nc.vector.tensor_copy(STtb[bi][:], tps[:, : 8 * K])

            def group(g, defer_acc=False):
                """Col group g: G matmuls + Exp per chunk pair; acc matmuls
                accumulate S^T E; fused DVE dot forms partials[:, g]."""
                fbv = FBq[g // 2][:, (g % 2) * 512 : (g % 2 + 1) * 512]
                chunks = list(range(NB - 1, 4 * g - 1, -1))
                pairs = [(chunks[i], chunks[i + 1]) for i in range(0, len(chunks), 2)]
                acc = accps.tile([K, 512], F32, tag="acc", name=f"acc{g}")
                deferred = []

                def acc_mms(pair, et):
                    for j, b in enumerate(pair):
                        nc.tensor.matmul(
                            acc[:],
                            STtb[b // 8][:, (b % 8) * K : (b % 8 + 1) * K],
                            et[:, j * 512 : (j + 1) * 512],
                            start=(b == NB - 1), stop=(b == 4 * g),
                        )

                for pi, pair in enumerate(pairs):
                    gt = gps.tile([128, 1024], F32, tag="g", name=f"g{g}_{pi}")
                    for j, b in enumerate(pair):
                        nc.tensor.matmul(
                            gt[:, j * 512 : (j + 1) * 512],
                            FAq[b // 8][:, (b % 8) * 128 : (b % 8 + 1) * 128],
                            fbv,
                            start=True, stop=True,
                        )
                    et = ep.tile([128, 1024], BF16, tag="e", name=f"e{g}_{pi}")
                    diag = pair[0] < 4 * g + 4
                    nc.scalar.activation(et[:], gt[:], AF.Exp, bias=0.0 if diag else bln2[:])
                    if defer_acc:
                        deferred.append((pair, et))
                    else:
                        acc_mms(pair, et)
                return acc, deferred, acc_mms

            def dot(g, acc):
                dsc = dscp.tile([K, 512], F32, tag="dsc", name=f"dsc{g}")
                nc.vector.tensor_mul(dsc[:], acc[:], Srow_y[g][:])
                nc.vector.tensor_reduce(
                    partials[:, g : g + 1], dsc[:], mybir.AxisListType.X, ALU.add
                )

            # ---- emission schedule ----
            feat_pre(3, "dve")
            feat_post(3, "act")
            at_stage()
            at3 = At[:, :].rearrange("x (k y) -> x k y", k=K, y=64)
            feat_pre(2, "act")
            feat_post(2, "dve")

            acc7, def7, accm7 = group(7, defer_acc=True)
            acc6, def6, accm6 = group(6, defer_acc=True)
            srow_stage([7, 6])
            stt_stage(3)
            for pair, et in def7:
                accm7(pair, et)
            dot(7, acc7)
            for pair, et in def6:
                accm6(pair, et)
            dot(6, acc6)

            acc5, def5, accm5 = group(5, defer_acc=True)
            srow_stage([5, 4])
            stt_stage(2)
            for pair, et in def5:
                accm5(pair, et)
            dot(5, acc5)

            acc4, _, _ = group(4)
            dot(4, acc4)
            late_skels()
            feat_pre(1, "pool")
            feat_post(1, "pool")
            srow_stage([3, 2])
            stt_stage(1)
            acc3, _, _ = group(3)
            dot(3, acc3)
            feat_pre(0, "pool")
            feat_post(0, "pool")
            srow_stage([1, 0])
            stt_stage(0)
            acc2, _, _ = group(2)
            dot(2, acc2)
            acc1, _, _ = group(1)
            dot(1, acc1)
            acc0, _, _ = group(0)
            dot(0, acc0)

            # ---- tail: sum partials, all-reduce over classes, scale ----
            nc.vector.tensor_reduce(pr1[:], partials[:], mybir.AxisListType.X, ALU.add)
            nc.gpsimd.partition_all_reduce(tot[:], pr1[:], K, bass_isa.ReduceOp.add)
            nc.vector.tensor_scalar_mul(osb[:], tot[0:1, :], float(-WEIGHT / N))
            nc.sync.dma_start(out_d[:], osb[:])

    nc.finalize()
    return nc


_CACHE = {}


def _get_nc():
    if "nc" not in _CACHE:
        _CACHE["nc"] = _build()
    return _CACHE["nc"]


def kernel(images: np.ndarray, segmentations: np.ndarray) -> np.ndarray:
    images = np.ascontiguousarray(np.asarray(images, dtype=np.float32))
    segmentations = np.ascontiguousarray(np.asarray(segmentations, dtype=np.float32))
    assert images.shape == (N, C, H, W) and segmentations.shape == (N, K, 128, 128)
    nc = _get_nc()
    consts = _consts()
    in_maps = [
        {"images": images[n].reshape(IMG_SHAPE), "segmentations": segmentations[n], **consts}
        for n in range(N)
    ]
    res = run_bass_kernel_spmd(nc, in_maps, list(range(N)))
    total = sum(float(res.results[n]["out"][0]) for n in range(N))
    return np.array([total], dtype=np.float32)


if __name__ == "__main__":
    rng = np.random.RandomState(0)
    img = rng.rand(N, C, H, W).astype(np.float32) * 255.0
    seg = rng.rand(N, K, 128, 128).astype(np.float32)
    print(kernel(img, seg))


# revision 25
# speedup vs baseline: 1.7358x; 1.0112x over previous
"""DenseCRF loss kernel for Trainium2, data-parallel over batch on 8 NeuronCores.

reference:
  seg = bilinear_resize(segmentations, 128->64)            # [N,K,64,64]
  f_i = [x_i/50, y_i/50, r_i/15, g_i/15, b_i/15]           # 5-dim bilateral feature
  W_ij = exp(-0.5*|f_i - f_j|^2)                           # [P,P], P=4096
  loss = WEIGHT * (-sum_k s_k^T W s_k) / N

Per core (1 image). W is symmetric, so only the lower triangle at 512x512
block granularity is computed: col group g (512 cols) contracts row chunks
b >= 4g. Off-diagonal blocks count twice -- the x2 rides the Exp bias as an
exact fp32 +ln2 (exp(G+ln2) = 2 exp(G)).

G(i,j) = f_i.f_j - q_i - q_j (q = 0.5|f|^2) is one 24-row bf16 matmul:
features split hi/lo (products exact in fp32 PSUM), and BOTH -q_i and -q_j
ride hi/lo bf16 row pairs, so Exp needs no data bias and can batch any pair
of PSUM banks. Exp'd blocks (bf16) contract against the resized segmentation
with PSUM accumulation per col group; a fused DVE tensor_tensor_reduce forms
per-group partials; host sums 8 cores.

Row layout of FA/FB [28, P] (G += sum_r FA[r,i]*FB[r,j]):
  0-2  (ch,ch)  3-5 (ch,cl)  6-8 (cl,ch)  9-11 (cl,cl)     colors hi/lo
  12-13 FA=(-qch,-qcl) FB=1  14-15 FA=1 FB=(-qch,-qcl)     color-q rows
  16-17 (ph,ph) 18-19 (ph,pl) 20-21 (pl,ph) 22-23 (pl,pl)  positions (const)
  24-25 FA=(-qph,-qpl) FB=1  26-27 FA=1 FB=(-qph,-qpl)     position-q (const)
"""

import sys

sys.path.insert(0, "/opt/trn_rl_repo")

import numpy as np
import ml_dtypes

import concourse.bass as bass
import concourse.tile as tile
from concourse import bacc, bass_isa, mybir
from concourse.bass_utils import run_bass_kernel_spmd

F32 = mybir.dt.float32
F32R = mybir.dt.float32r
BF16 = mybir.dt.bfloat16
AF = mybir.ActivationFunctionType
ALU = mybir.AluOpType
BF = ml_dtypes.bfloat16

N, C, K = 8, 3, 21
H, W = 64, 64
P = H * W  # 4096
SIGMA_RGB = 15.0
SXY = 100.0 * 0.5  # sigma_xy * scale
WEIGHT = 1e-8
NB = 32  # 128-row chunks of P
NG = 8  # 512-col groups of P
NQ = 4  # 1024-col quarters (feature prep granularity)
LN2 = float(np.log(2.0))
KA, KB = 11, 10  # seg class split across the two load DMAs
IMG_SHAPE = (C, 32, 128)  # img DRAM layout: 512B runs give sane DMA descriptors


def _resize_matrix():
    """[64,128] weights of jax.image.resize(..., method='bilinear') along one dim
    (triangle kernel, antialias=True, scale=0.5, renormalized)."""
    y = np.arange(128, dtype=np.float64)[:, None]
    sample = 2.0 * np.arange(64, dtype=np.float64)[None, :] + 0.5
    w = np.maximum(0.0, 1.0 - 0.5 * np.abs(y - sample))
    w = w / w.sum(axis=0, keepdims=True)
    return np.ascontiguousarray(w.T.astype(np.float32))  # [64,128]


def _consts():
    R = _resize_matrix()  # [64,128]
    rtf = np.ascontiguousarray(R.T)  # [128,64] f32
    rtb = rtf.astype(BF)
    idf = np.eye(K, dtype=np.float32)
    i = np.arange(P, dtype=np.float32)
    px = (i % 64).astype(np.float32) / np.float32(SXY)
    py = (i // 64).astype(np.float32) / np.float32(SXY)
    pos = np.stack([px, py])  # [2,P] f32
    ph2 = pos.astype(BF)
    pl2 = (pos - ph2.astype(np.float32)).astype(BF)
    pf2 = ph2.astype(np.float64) + pl2.astype(np.float64)  # exact f~ positions
    qpos = -0.5 * (pf2[0] ** 2 + pf2[1] ** 2)  # [P] f64
    qph = qpos.astype(np.float32).astype(BF)
    qpl = (qpos - qph.astype(np.float64)).astype(np.float32).astype(BF)
    # constant skeleton rows 12..27 of FA/FB (zeros where color-q rows land)
    skA = np.zeros((16, P), dtype=BF)
    skB = np.zeros((16, P), dtype=BF)
    skA[2:4] = 1.0
    skB[0:2] = 1.0
    skA[4:6] = ph2
    skA[6:8] = ph2
    skA[8:10] = pl2
    skA[10:12] = pl2
    skB[4:6] = ph2
    skB[6:8] = pl2
    skB[8:10] = ph2
    skB[10:12] = pl2
    skA[12], skA[13], skA[14], skA[15] = qph, qpl, 1.0, 1.0
    skB[12], skB[13], skB[14], skB[15] = 1.0, 1.0, qph, qpl
    return dict(rtf=rtf, rtb=rtb, idf=idf, fabA=skA, fabB=skB)


def _build():
    nc = bacc.Bacc()
    images_d = nc.dram_tensor("images", list(IMG_SHAPE), F32, kind="ExternalInput")
    seg_d = nc.dram_tensor("segmentations", [K, 128, 128], F32, kind="ExternalInput")
    rtf_d = nc.dram_tensor("rtf", [128, 64], F32, kind="ExternalInput")
    rtb_d = nc.dram_tensor("rtb", [128, 64], BF16, kind="ExternalInput")
    idf_d = nc.dram_tensor("idf", [K, K], F32, kind="ExternalInput")
    fabA_d = nc.dram_tensor("fabA", [16, P], BF16, kind="ExternalInput")
    fabB_d = nc.dram_tensor("fabB", [16, P], BF16, kind="ExternalInput")
    out_d = nc.dram_tensor("out", [1], F32, kind="ExternalOutput")

    inv15 = float(np.float32(1.0) / np.float32(SIGMA_RGB))
    inv225 = float(np.float32(inv15) * np.float32(inv15))

    with tile.TileContext(nc) as tc:
        with (
            tc.tile_pool(name="persist", bufs=1) as pp,
            tc.tile_pool(name="rp", bufs=2, space="PSUM") as rp,
            tc.tile_pool(name="gps", bufs=2, space="PSUM") as gps,
            tc.tile_pool(name="accps", bufs=2, space="PSUM") as accps,
            tc.tile_pool(name="ep", bufs=10) as ep,
            tc.tile_pool(name="dscp", bufs=2) as dscp,
        ):
            FAq = [pp.tile([28, 1024], BF16, tag=f"FA{q}", name=f"FA{q}") for q in range(NQ)]
            FBq = [pp.tile([28, 1024], BF16, tag=f"FB{q}", name=f"FB{q}") for q in range(NQ)]
            img_s = pp.tile([C, P], F32, tag="img")
            seg_a = pp.tile([128, KA * 128], F32, tag="sega")
            seg_b = pp.tile([128, KB * 128], F32, tag="segb")
            rtf_s = pp.tile([128, 64], F32, tag="rtf")
            rtb_s = pp.tile([128, 64], BF16, tag="rtb")
            idf_s = pp.tile([K, K], F32, tag="idf")
            fsqq = [pp.tile([C, 1024], F32, tag=f"fsq{q}", name=f"fsq{q}") for q in range(NQ)]
            q3q = [pp.tile([C, 1024], F32, tag=f"q3{q}", name=f"q3{q}") for q in range(NQ)]
            cstq = [pp.tile([64, 1024], BF16, tag=f"cst{q}", name=f"cst{q}") for q in range(NQ)]
            cst2q = [pp.tile([64, 1024], BF16, tag=f"cs2{q}", name=f"cs2{q}") for q in range(NQ)]
            qstq = [pp.tile([64, 1024], BF16, tag=f"qst{q}", name=f"qst{q}") for q in range(NQ)]
            At = pp.tile([128, K * 64], BF16, tag="At")
            Srow_y = [pp.tile([K, 512], F32, tag=f"sr{y}", name=f"sr{y}") for y in range(NG)]
            STtb = [pp.tile([128, 8 * K], BF16, tag=f"stt{i}", name=f"stt{i}") for i in range(4)]
            partials = pp.tile([K, NG], F32, tag="partials")
            pr1 = pp.tile([K, 1], F32, tag="pr1")
            tot = pp.tile([K, 1], F32, tag="tot")
            osb = pp.tile([1, 1], F32, tag="osb")
            bln2 = pp.tile([128, 1], F32, tag="bln2")

            qS = nc.sync.dma_start
            qP = nc.gpsimd.dma_start
            qA = nc.scalar.dma_start

            # ---- input loads ----
            # SP: img first (gates the feature chain), then seg half A +
            # Q3 skeletons + resize consts. Pool: seg half B issued early
            # (transfer overlaps). Act: Q2 skeletons (idle early).
            nc.gpsimd.memset(bln2[:], LN2)
            qP(img_s[:], images_d[:])  # [C,32,128] -> [C,P]
            segr = seg_d.rearrange("k y x -> y k x")
            qS(seg_a[:], segr[:, :KA, :])
            qS(rtf_s[:], rtf_d[:])
            qP(seg_b[:], segr[:, KA:, :])
            qA(FAq[3][12:28, :], fabA_d[:, 3 * 1024 : 4 * 1024])
            qA(FBq[3][12:28, :], fabB_d[:, 3 * 1024 : 4 * 1024])
            qA(FAq[2][12:28, :], fabA_d[:, 2 * 1024 : 3 * 1024])
            qA(FBq[2][12:28, :], fabB_d[:, 2 * 1024 : 3 * 1024])
            qA(rtb_s[:], rtb_d[:])
            qA(idf_s[:], idf_d[:])

            def late_skels():
                for q in (1, 0):
                    sel = slice(q * 1024, (q + 1) * 1024)
                    qS(FAq[q][12:28, :], fabA_d[:, sel])
                    qA(FBq[q][12:28, :], fabB_d[:, sel])

            def feat_pre(q, fh_eng="act"):
                """Colors hi/lo for quarter q staged into cstq[q] quadrants
                (fh at 0, fh-copy at 32, fl at 64, fl-copy at 96), then ONE
                DMA each into FA[0:12] / FB[0:12] via strided partition APs.
                Pairing: FA rows = (h, h', l, l'), FB rows = (h, l, h', l')
                -> combos (h,h),(h,l),(l,h),(l,l)."""
                sel = slice(q * 1024, (q + 1) * 1024)
                cst, fsq = cstq[q], fsqq[q]
                fh, fl = cst[0:3, :], cst[32:35, :]
                if fh_eng == "act":
                    nc.scalar.activation(fh, img_s[:, sel], AF.Copy, scale=inv15)
                elif fh_eng == "dve":
                    nc.vector.tensor_scalar_mul(fh, img_s[:, sel], inv15)
                else:
                    nc.gpsimd.tensor_scalar_mul(fh, img_s[:, sel], inv15)
                nc.vector.scalar_tensor_tensor(
                    fsq[:], img_s[:, sel], inv225, img_s[:, sel], ALU.mult, ALU.mult
                )
                nc.vector.scalar_tensor_tensor(
                    fl, img_s[:, sel], inv15, fh, ALU.mult, ALU.subtract
                )
                FA, FB = FAq[q], FBq[q]
                moves = [
                    (FA[0:3, :], fh), (FB[0:3, :], fh),
                    (FA[6:9, :], fl), (FB[3:6, :], fl),
                    (FA[3:6, :], fh), (FB[6:9, :], fh),
                    (FA[9:12, :], fl), (FB[9:12, :], fl),
                ]
                for i, (dst, srct) in enumerate(moves):
                    [qS, qP][i % 2](dst, srct[:])

            def feat_post(q, qh_eng="act"):
                """color-q rows (-0.5|c|^2 hi/lo) staged into qstq[q]
                (qh at 0, ql at 32), one DMA each into FA[12:14]/FB[14:16]."""
                q3, qst = q3q[q], qstq[q]
                qh, ql = qst[0:1, :], qst[32:33, :]
                nc.gpsimd.partition_all_reduce(q3[:], fsqq[q][:], C, bass_isa.ReduceOp.add)
                if qh_eng == "act":
                    nc.scalar.activation(qh, q3[0:1, :], AF.Copy, scale=-0.5)
                elif qh_eng == "dve":
                    nc.vector.tensor_scalar_mul(qh, q3[0:1, :], -0.5)
                else:
                    nc.gpsimd.tensor_scalar_mul(qh, q3[0:1, :], -0.5)
                nc.vector.scalar_tensor_tensor(
                    ql, q3[0:1, :], -0.5, qh, ALU.mult, ALU.subtract
                )
                for i, (dst, srct) in enumerate([
                    (FAq[q][12:13, :], qh), (FBq[q][14:15, :], qh),
                    (FAq[q][13:14, :], ql), (FBq[q][15:16, :], ql),
                ]):
                    [qS, qP][i % 2](dst, srct)

            def at_stage():
                # At[x, k*64+y'] = sum_y seg[y,(k,x)] * rtf[y,y']  (f32r)
                for k0 in range(0, K, 8):
                    k1 = min(k0 + 8, K)
                    aps = rp.tile([128, 512], F32, tag="rp", name=f"at{k0}")
                    for k in range(k0, k1):
                        src = seg_a if k < KA else seg_b
                        koff = k if k < KA else k - KA
                        nc.tensor.matmul(
                            aps[:, (k - k0) * 64 : (k - k0 + 1) * 64],
                            src[:, koff * 128 : (koff + 1) * 128],
                            rtf_s[:],
                            start=True, stop=True,
                        )
                    nc.vector.tensor_copy(At[:, k0 * 64 : k1 * 64], aps[:, : (k1 - k0) * 64])

            at3 = None

            def srow_stage(ybs):
                # Srow[k, yb*512 + yl*64 + x'] = sum_x At[x,(k,y')] * rtb[x,x']
                for yb in ybs:
                    sps = rp.tile([128, 512], F32, tag="rp", name=f"sr{yb}")
                    for yl in range(8):
                        yp = yb * 8 + yl
                        nc.tensor.matmul(
                            sps[0:K, yl * 64 : (yl + 1) * 64],
                            at3[:, :, yp], rtb_s[:],
                            start=True, stop=True,
                        )
                    nc.vector.tensor_copy(Srow_y[yb][:], sps[0:K, :])

            def stt_stage(bi):
                # STt chunks for batch bi: chunks 8*bi .. 8*bi+7
                tps = rp.tile([128, 512], F32, tag="rp", name=f"st{bi}")
                for j in range(8):
                    b = 8 * bi + j
                    yb, rest = divmod(b * 128, 512)
                    nc.tensor.transpose(
                        tps[:, j * K : (j + 1) * K],
                        Srow_y[yb][:, rest : rest + 128],
                        idf_s[:],
                    )
                nc.vector.tensor_copy(STtb[bi][:], tps[:, : 8 * K])

            def group(g, defer_acc=False):
                """Col group g: G matmuls + Exp per chunk pair; acc matmuls
                accumulate S^T E; fused DVE dot forms partials[:, g]."""
                fbv = FBq[g // 2][:, (g % 2) * 512 : (g % 2 + 1) * 512]
                chunks = list(range(NB - 1, 4 * g - 1, -1))
                pairs = [(chunks[i], chunks[i + 1]) for i in range(0, len(chunks), 2)]
                acc = accps.tile([K, 512], F32, tag="acc", name=f"acc{g}")
                deferred = []

                def acc_mms(pair, et):
                    for j, b in enumerate(pair):
                        nc.tensor.matmul(
                            acc[:],
                            STtb[b // 8][:, (b % 8) * K : (b % 8 + 1) * K],
                            et[:, j * 512 : (j + 1) * 512],
                            start=(b == NB - 1), stop=(b == 4 * g),
                        )

                for pi, pair in enumerate(pairs):
                    gt = gps.tile([128, 1024], F32, tag="g", name=f"g{g}_{pi}")
                    for j, b in enumerate(pair):
                        nc.tensor.matmul(
                            gt[:, j * 512 : (j + 1) * 512],
                            FAq[b // 8][:, (b % 8) * 128 : (b % 8 + 1) * 128],
                            fbv,
                            start=True, stop=True,
                        )
                    et = ep.tile([128, 1024], BF16, tag="e", name=f"e{g}_{pi}")
                    diag = pair[0] < 4 * g + 4
                    nc.scalar.activation(et[:], gt[:], AF.Exp, bias=0.0 if diag else bln2[:])
                    if defer_acc:
                        deferred.append((pair, et))
                    else:
                        acc_mms(pair, et)
                return acc, deferred, acc_mms

            def dot(g, acc):
                dsc = dscp.tile([K, 512], F32, tag="dsc", name=f"dsc{g}")
                nc.vector.tensor_mul(dsc[:], acc[:], Srow_y[g][:])
                nc.vector.tensor_reduce(
                    partials[:, g : g + 1], dsc[:], mybir.AxisListType.X, ALU.add
                )

            # ---- emission schedule ----
            feat_pre(3, "dve")
            feat_post(3, "act")
            at_stage()
            at3 = At[:, :].rearrange("x (k y) -> x k y", k=K, y=64)
            feat_pre(2, "act")
            feat_post(2, "dve")

            acc7, def7, accm7 = group(7, defer_acc=True)
            acc6, def6, accm6 = group(6, defer_acc=True)
            srow_stage([7, 6])
            stt_stage(3)
            for pair, et in def7:
                accm7(pair, et)
            dot(7, acc7)
            for pair, et in def6:
                accm6(pair, et)
            dot(6, acc6)

            acc5, def5, accm5 = group(5, defer_acc=True)
            srow_stage([5, 4])
            stt_stage(2)
            for pair, et in def5:
                accm5(pair, et)
            dot(5, acc5)

            acc4, _, _ = group(4)
            dot(4, acc4)
            late_skels()
            feat_pre(1, "pool")
            feat_post(1, "pool")
            srow_stage([3, 2])
            stt_stage(1)
            acc3, _, _ = group(3)
            dot(3, acc3)
            feat_pre(0, "pool")
            feat_post(0, "pool")
            srow_stage([1, 0])
            stt_stage(0)
            acc2, _, _ = group(2)
            dot(2, acc2)
            acc1, _, _ = group(1)
            dot(1, acc1)
            acc0, _, _ = group(0)
            dot(0, acc0)

            # ---- tail: sum partials, all-reduce over classes, scale ----
            nc.vector.tensor_reduce(pr1[:], partials[:], mybir.AxisListType.X, ALU.add)
            nc.gpsimd.partition_all_reduce(tot[:], pr1[:], K, bass_isa.ReduceOp.add)
            nc.vector.tensor_scalar_mul(osb[:], tot[0:1, :], float(-WEIGHT / N))
            nc.sync.dma_start(out_d[:], osb[:])

    nc.finalize()
    return nc


_CACHE = {}


def _get_nc():
    if "nc" not in _CACHE:
        _CACHE["nc"] = _build()
    return _CACHE["nc"]


def kernel(images: np.ndarray, segmentations: np.ndarray) -> np.ndarray:
    images = np.ascontiguousarray(np.asarray(images, dtype=np.float32))
    segmentations = np.ascontiguousarray(np.asarray(segmentations, dtype=np.float32))
    assert images.shape == (N, C, H, W) and segmentations.shape == (N, K, 128, 128)
    nc = _get_nc()
    consts = _consts()
    in_maps = [
        {"images": images[n].reshape(IMG_SHAPE), "segmentations": segmentations[n], **consts}
        for n in range(N)
    ]
    res = run_bass_kernel_spmd(nc, in_maps, list(range(N)))
    total = sum(float(res.results[n]["out"][0]) for n in range(N))
    return np.array([total], dtype=np.float32)


if __name__ == "__main__":
    rng = np.random.RandomState(0)
    img = rng.rand(N, C, H, W).astype(np.float32) * 255.0
    seg = rng.rand(N, K, 128, 128).astype(np.float32)
    print(kernel(img, seg))
